# revision 12
# baseline (speedup 1.0000x reference)
"""Trainium2 Bass kernel for the CNF (continuous normalizing flow) problem.

reference math: RK4 integration (8 steps) of
    dz/dt = f(t,z) = MLP(concat[z, t]),  dlogp/dt = -tr(df/dz)
with MLP = tanh(W1x+b1) -> tanh(W2h+b2) -> W3h+b3, N=16384, D=8, H=128.

Key algebra (validated to fp32 accuracy on host):
  - exact Jacobian trace as a bilinear form:  tr = u^T C v  with
    u = 1-h1^2, v = 1-h2^2, C = W2 * (W1[:8].T @ W3.T)  (constant).
  - tr = S0 - c_rowsum.h1sq + sum_m[(P - c_colsum) * h2sq],  P = C^T h1sq,
    so only two partition-reduce matmuls (+ a fused DVE scalar_tensor_tensor)
    are needed per eval; the S0 constant is folded in on the host.
  - RK4 intermediate states are never materialized: the layer-1 preact for
    eval e is accumulated in PSUM as W1z^T z + c_e (W3@W1z)^T h2_{e-1},
    with (b1 + t*W1[8] + c_e W1z^T b3) applied as the tanh per-partition bias.

Layout: feature-major [features->partitions, samples->free]. Data-parallel
over N across 8 cores; per core 2048 samples = 4 chunks of 512 (PSUM bank
width). State z lives packed in one [128,512] tile, chunk c at partition
strip 32c (8 rows used per strip), enabling row/col-tiled concurrent small
matmuls and full-width elementwise ops for the tiny D=8 state.
"""

import numpy as np
import ml_dtypes

import concourse.bass as bass
import concourse.bacc as bacc
import concourse.tile as tile
import concourse.mybir as mybir
from concourse.bass_utils import run_bass_kernel_spmd

F32 = mybir.dt.float32
F32R = mybir.dt.float32r
F16 = mybir.dt.float16
AF = mybir.ActivationFunctionType
OP = mybir.AluOpType

N, D, H, T = 16384, 8, 128, 9
NCORES = 8
NSAMP = N // NCORES          # 2048 samples per core
S = 512                      # chunk width (one PSUM bank of fp32)
NCH = NSAMP // S             # 4 chunks per core
NSTEP = T - 1                # 8 RK4 steps

_CACHE = {}


def _build_nc():
    nc = bacc.Bacc("TRN2", target_bir_lowering=False, debug=False)

    din = {}
    din["z0p"] = nc.dram_tensor("z0p", [128, S], F32, kind="ExternalInput")
    din["w1zrep"] = nc.dram_tensor("w1zrep", [128, H], F16, kind="ExternalInput")
    din["gmat"] = nc.dram_tensor("gmat", [2, H, H], F16, kind="ExternalInput")
    din["w2"] = nc.dram_tensor("w2", [H, H], F16, kind="ExternalInput")
    din["cmat"] = nc.dram_tensor("cmat", [H, H], F16, kind="ExternalInput")
    din["w3g"] = nc.dram_tensor("w3g", [H, 2 * D], F16, kind="ExternalInput")
    din["rvec"] = nc.dram_tensor("rvec", [128, 4], F16, kind="ExternalInput")
    din["ccol"] = nc.dram_tensor("ccol", [128, 1], F32, kind="ExternalInput")
    din["btab"] = nc.dram_tensor("btab", [128, NSTEP * 4], F32, kind="ExternalInput")
    din["b2col"] = nc.dram_tensor("b2col", [128, 1], F32, kind="ExternalInput")
    ztp = nc.dram_tensor("ztp", [NSTEP, 128, S], F32, kind="ExternalOutput")
    lpp = nc.dram_tensor("lpp", [NSTEP, NCH, S], F32, kind="ExternalOutput")

    with tile.TileContext(nc) as tc:
        _body(nc, tc, din, ztp, lpp)
    nc.compile()
    return nc


def _body(nc, tc, din, ztp, lpp):
    with (
        tc.tile_pool(name="const", bufs=1) as const,
        tc.tile_pool(name="state", bufs=1) as state,
        tc.tile_pool(name="h1", bufs=8) as h1p,
        tc.tile_pool(name="h1sq", bufs=8) as h1sqp,
        tc.tile_pool(name="h2", bufs=8) as h2p,
        tc.tile_pool(name="h2sq", bufs=4) as h2sqp,
        tc.tile_pool(name="pw", bufs=4) as pwp,
        tc.tile_pool(name="apsum", bufs=4, space="PSUM") as apool,
        tc.tile_pool(name="ppsum", bufs=2, space="PSUM") as ppool,
        tc.tile_pool(name="dpsum", bufs=1, space="PSUM") as dpool,
        tc.tile_pool(name="lpsum", bufs=1, space="PSUM") as lpool,
    ):
        def load(name, shape, dtype, src):
            t = const.tile(shape, dtype, tag=name, name=name)
            nc.sync.dma_start(t[:], src)
            return t

        w1z = load("w1z", [128, H], F16, din["w1zrep"][:, :])
        g = [load(f"g{i}", [H, H], F16, din["gmat"][i, :, :]) for i in range(2)]
        w2 = load("w2", [H, H], F16, din["w2"][:, :])
        cm = load("cm", [H, H], F16, din["cmat"][:, :])
        w3g = load("w3g", [H, 2 * D], F16, din["w3g"][:, :])
        rvec = load("rvec", [128, 4], F16, din["rvec"][:, :])
        ccol = load("ccol", [128, 1], F32, din["ccol"][:, :])
        btab = load("btab", [128, NSTEP * 4], F32, din["btab"][:, :])
        b2col = load("b2col", [128, 1], F32, din["b2col"][:, :])

        zpk = [state.tile([128, S], F32, tag=f"z{i}", name=f"z{i}") for i in range(2)]
        lpk = [state.tile([128, S], F32, tag=f"l{i}", name=f"l{i}") for i in range(2)]
        zf16 = [state.tile([128, S], F16, tag=f"zf{i}", name=f"zf{i}") for i in range(2)]
        nc.sync.dma_start(zpk[0][:], din["z0p"][:, :])
        nc.vector.memset(lpk[0][:], 0.0)
        nc.vector.tensor_copy(zf16[0][:], zpk[0][:])

        # gamma index per eval: dt/6 for evals 0,3 ; dt/3 for evals 1,2
        gidx = [0, 1, 1, 0]

        # persistent PSUM accumulators (strip rows are MM-written each step;
        # the other rows stay at the memset value so full-tile reads are defined)
        zdelta = dpool.tile([128, S], F32, tag="zd", name="zd")
        lpdelta = lpool.tile([128, S], F32, tag="lpd", name="lpd")
        nc.vector.memset(zdelta[:], 0.0)
        nc.vector.memset(lpdelta[:], 0.0)

        for s in range(NSTEP):
            zsrc, zdst = zpk[s % 2], zpk[(s + 1) % 2]
            lsrc, ldst = lpk[s % 2], lpk[(s + 1) % 2]
            h2_prev = None
            for e in range(4):
                bcol = s * 4 + e
                a1_t, h1_t, a2_t, h1sq_t, h2_t, p_t, h2sq_t, pw_t = ([] for _ in range(8))
                for c in range(NCH):
                    a1 = apool.tile([128, S], F32, tag="a")
                    nc.tensor.matmul(
                        a1[:],
                        w1z[32 * c : 32 * c + D, :],
                        zf16[s % 2][32 * c : 32 * c + D, :],
                        start=True,
                        stop=(e == 0),
                        tile_position=(32 * c, 0),
                    )
                    if e > 0:
                        # zvar coefficient: evals 1,2 use dt/2 (g[0]); eval 3 uses dt (g[1])
                        nc.tensor.matmul(
                            a1[:], g[0 if e < 3 else 1][:], h2_prev[c][:], start=False, stop=True
                        )
                    a1_t.append(a1)
                for c in range(NCH):
                    h1 = h1p.tile([128, S], F16, tag="h1")
                    nc.scalar.activation(
                        h1[:], a1_t[c][:], AF.Tanh, bias=btab[:, bcol : bcol + 1]
                    )
                    h1_t.append(h1)
                for c in range(NCH):
                    a2 = apool.tile([128, S], F32, tag="a")
                    nc.tensor.matmul(a2[:], w2[:], h1_t[c][:], start=True, stop=True)
                    a2_t.append(a2)
                for c in range(NCH):
                    h1sq = h1sqp.tile([128, S], F16, tag="h1sq")
                    nc.gpsimd.tensor_mul(h1sq[:], h1_t[c][:], h1_t[c][:])
                    h1sq_t.append(h1sq)
                for c in range(NCH):
                    h2 = h2p.tile([128, S], F16, tag="h2")
                    nc.scalar.activation(
                        h2[:], a2_t[c][:], AF.Tanh, bias=b2col[:, 0:1]
                    )
                    h2_t.append(h2)
                for c in range(NCH):
                    p = ppool.tile([128, S], F32, tag="p")
                    nc.tensor.matmul(p[:], cm[:], h1sq_t[c][:], start=True, stop=True)
                    p_t.append(p)
                for c in range(NCH):
                    h2sq = h2sqp.tile([128, S], F16, tag="h2sq")
                    nc.vector.tensor_mul(h2sq[:], h2_t[c][:], h2_t[c][:])
                    h2sq_t.append(h2sq)
                for c in range(NCH):
                    pw = pwp.tile([128, S], F16, tag="pw")
                    nc.vector.scalar_tensor_tensor(
                        pw[:],
                        p_t[c][:],
                        ccol[:, 0:1],
                        h2sq_t[c][:],
                        op0=OP.subtract,
                        op1=OP.mult,
                    )
                    pw_t.append(pw)
                gi = gidx[e]
                for c in range(NCH):
                    nc.tensor.matmul(
                        zdelta[32 * c : 32 * c + D, :],
                        w3g[:, D * gi : D * gi + D],
                        h2_t[c][:],
                        start=(e == 0),
                        stop=(e == 3),
                        tile_position=(0, 32 * c),
                        skip_group_check=True,
                    )
                for c in range(NCH):
                    nc.tensor.matmul(
                        lpdelta[32 * c : 32 * c + 1, :],
                        rvec[:, gi : gi + 1],
                        h1sq_t[c][:],
                        start=(e == 0),
                        stop=False,
                        tile_position=(0, 32 * c),
                        skip_group_check=True,
                    )
                    nc.tensor.matmul(
                        lpdelta[32 * c : 32 * c + 1, :],
                        rvec[:, 2 + gi : 3 + gi],
                        pw_t[c][:],
                        start=False,
                        stop=(e == 3),
                        tile_position=(0, 32 * c),
                        skip_group_check=True,
                    )
                h2_prev = h2_t
            nc.vector.tensor_add(zdst[:], zsrc[:], zdelta[:])
            nc.vector.tensor_copy(zf16[(s + 1) % 2][:], zdst[:])
            nc.vector.tensor_add(ldst[:], lsrc[:], lpdelta[:])
            nc.sync.dma_start(ztp[s, :, :], zdst[:])
            for c in range(NCH):
                nc.sync.dma_start(lpp[s, c : c + 1, :], ldst[32 * c : 32 * c + 1, :])


def _host_prep(ts, z0, logp_diff_t0, W1, b1, W2, b2, W3, b3):
    """Precompute all per-core input tensors (float64 internally)."""
    f8 = np.float64
    ts = ts.astype(f8)
    W1, b1 = W1.astype(f8), b1.astype(f8)
    W2, b2 = W2.astype(f8), b2.astype(f8)
    W3, b3 = W3.astype(f8), b3.astype(f8)
    dts = np.diff(ts)
    assert np.allclose(dts, dts[0], rtol=0, atol=1e-9), "nonuniform ts unsupported"
    dt = float(dts[0])

    W1z = W1[:D, :]
    w1t = W1[D, :]
    B = W1z.T @ W3.T
    C = W2 * B
    G = W3 @ W1z
    c_rowsum = C.sum(axis=1)
    c_colsum = C.sum(axis=0)
    S0 = float(C.sum())
    ga, gb = dt / 6.0, dt / 3.0
    c_coefs = [0.0, dt / 2.0, dt / 2.0, dt]

    bf = np.float16
    w1zrep = np.zeros((128, H), np.float16)
    for c in range(NCH):
        w1zrep[32 * c : 32 * c + D, :] = W1z
    gmat = np.stack([(dt / 2.0) * G, dt * G]).astype(bf)        # [2,H,H]
    w3g = np.concatenate([ga * W3, gb * W3], axis=1).astype(bf)  # [H,16]
    rvec = np.stack(
        [ga * c_rowsum, gb * c_rowsum, -ga * np.ones(H), -gb * np.ones(H)], axis=1
    ).astype(bf)                                                 # [128,4]
    btab = np.zeros((128, NSTEP * 4), np.float32)
    w1tb3 = W1z.T @ b3
    for s in range(NSTEP):
        t0 = float(ts[s])
        tv = [t0, t0 + dt / 2.0, t0 + dt / 2.0, t0 + dt]
        for e in range(4):
            btab[:, s * 4 + e] = b1 + tv[e] * w1t + c_coefs[e] * w1tb3

    shared = dict(
        w1zrep=w1zrep,
        gmat=gmat,
        w2=W2.astype(bf),
        cmat=C.astype(bf),
        w3g=w3g,
        rvec=rvec,
        ccol=c_colsum.astype(np.float32).reshape(128, 1),
        btab=btab,
        b2col=b2.astype(np.float32).reshape(128, 1),
    )
    in_maps = []
    z0 = np.asarray(z0, np.float32)
    for core in range(NCORES):
        z0p = np.zeros((128, S), np.float32)
        for c in range(NCH):
            seg = z0[core * NSAMP + c * S : core * NSAMP + (c + 1) * S, :]  # [S,D]
            z0p[32 * c : 32 * c + D, :] = seg.T
        in_maps.append(dict(shared, z0p=z0p))
    return in_maps, dt, S0


def _assemble(results, z0, logp_diff_t0, dt, S0):
    z0 = np.asarray(z0, np.float32)
    lp0 = np.asarray(logp_diff_t0, np.float32)
    zt = np.empty((T, N, D), np.float32)
    lp = np.empty((T, N, 1), np.float32)
    zt[0] = z0
    lp[0] = lp0
    for core in range(NCORES):
        ztp = results[core]["ztp"]  # [NSTEP,128,S]
        lpp = results[core]["lpp"]  # [NSTEP,NCH,S]
        for c in range(NCH):
            sl = slice(core * NSAMP + c * S, core * NSAMP + (c + 1) * S)
            zt[1:, sl, :] = ztp[:, 32 * c : 32 * c + D, :].transpose(0, 2, 1)
            lp[1:, sl, 0] = lpp[:, c, :]
    for s in range(1, T):
        lp[s] += lp0 - np.float32(s * dt * S0)
    return zt, lp


def kernel(ts, z0, logp_diff_t0, W1, b1, W2, b2, W3, b3, _run=None):
    if "nc" not in _CACHE:
        _CACHE["nc"] = _build_nc()
    nc = _CACHE["nc"]
    in_maps, dt, S0 = _host_prep(ts, z0, logp_diff_t0, W1, b1, W2, b2, W3, b3)
    if _run is None:
        results = run_bass_kernel_spmd(nc, in_maps, core_ids=list(range(NCORES))).results
    else:
        results = _run(nc, in_maps)
    return _assemble(results, z0, logp_diff_t0, dt, S0)


# revision 20
# speedup vs baseline: 2800.6221x; 2800.6221x over previous
"""Trainium2 Bass kernel for the CNF (continuous normalizing flow) problem.

reference math: RK4 integration (8 steps) of
    dz/dt = f(t,z) = MLP(concat[z, t]),  dlogp/dt = -tr(df/dz)
with MLP = tanh(W1x+b1) -> tanh(W2h+b2) -> W3h+b3, N=16384, D=8, H=128.

Key algebra (validated to fp32 accuracy on host):
  - exact Jacobian trace as a bilinear form:  tr = u^T C v  with
    u = 1-h1^2, v = 1-h2^2, C = W2 * (W1[:8].T @ W3.T)  (constant).
  - tr = S0 - c_rowsum.h1sq + sum_m[(P - c_colsum) * h2sq],  P = C^T h1sq,
    so only two partition-reduce matmuls (+ a fused DVE scalar_tensor_tensor)
    are needed per eval; the S0 constant is folded in on the host.
  - RK4 intermediate states are never materialized: the layer-1 preact for
    eval e is accumulated in PSUM as W1z^T z + c_e (W3@W1z)^T h2_{e-1},
    with (b1 + t*W1[8] + c_e W1z^T b3) applied as the tanh per-partition bias.

Layout: feature-major [features->partitions, samples->free]. Data-parallel
over N across 8 cores; per core 2048 samples = 4 chunks of 512 (PSUM bank
width). State z lives packed in one [128,512] tile, chunk c at partition
strip 32c (8 rows used per strip), enabling row/col-tiled concurrent small
matmuls and full-width elementwise ops for the tiny D=8 state.
"""

import numpy as np
import ml_dtypes

import concourse.bass as bass
import concourse.bacc as bacc
import concourse.tile as tile
import concourse.mybir as mybir
from concourse.bass_utils import run_bass_kernel_spmd

F32 = mybir.dt.float32
F32R = mybir.dt.float32r
F16 = mybir.dt.float16  # same PE/DVE speed as bf16, 8x more mantissa
AF = mybir.ActivationFunctionType
OP = mybir.AluOpType

N, D, H, T = 16384, 8, 128, 9
NCORES = 8
NSAMP = N // NCORES          # 2048 samples per core
S = 512                      # chunk width (one PSUM bank of fp32)
NCH = NSAMP // S             # 4 chunks per core
NSTEP = T - 1                # 8 RK4 steps

_CACHE = {}


def _build_nc(repeat=1):
    nc = bacc.Bacc("TRN2", target_bir_lowering=False, debug=False)

    din = {}
    din["z0p"] = nc.dram_tensor("z0p", [128, S], F32, kind="ExternalInput")
    din["w1zrep"] = nc.dram_tensor("w1zrep", [128, H], F16, kind="ExternalInput")
    din["gmat"] = nc.dram_tensor("gmat", [2, H, H], F16, kind="ExternalInput")
    din["w2"] = nc.dram_tensor("w2", [H, H], F16, kind="ExternalInput")
    din["cmat"] = nc.dram_tensor("cmat", [H, H], F16, kind="ExternalInput")
    din["w3g"] = nc.dram_tensor("w3g", [H, 2 * D], F16, kind="ExternalInput")
    din["rvec"] = nc.dram_tensor("rvec", [128, 4], F16, kind="ExternalInput")
    din["ccol"] = nc.dram_tensor("ccol", [128, 1], F32, kind="ExternalInput")
    din["btab"] = nc.dram_tensor("btab", [128, NSTEP * 4], F32, kind="ExternalInput")
    din["b2col"] = nc.dram_tensor("b2col", [128, 1], F32, kind="ExternalInput")
    ztp = nc.dram_tensor("ztp", [NSTEP, 128, S], F32, kind="ExternalOutput")
    lpp = nc.dram_tensor("lpp", [NSTEP, NCH, S], F32, kind="ExternalOutput")

    with tile.TileContext(nc) as tc:
        _body(nc, tc, din, ztp, lpp, repeat)
    nc.compile()
    return nc


def _body(nc, tc, din, ztp, lpp, repeat=1):
    # NB: GPSIMD tensor ops measured ~5-10x slower than DVE on HW - keep all
    # elementwise work on DVE (VectorE) and transcendentals on ScalarE.
    SQ1_ENGINE = nc.vector
    with (
        tc.tile_pool(name="const", bufs=1) as const,
        tc.tile_pool(name="state", bufs=1) as state,
        tc.tile_pool(name="h1", bufs=8) as h1p,
        tc.tile_pool(name="h1sq", bufs=8) as h1sqp,
        tc.tile_pool(name="h2", bufs=8) as h2p,
        tc.tile_pool(name="h2sq", bufs=4) as h2sqp,
        tc.tile_pool(name="pw", bufs=4) as pwp,
        tc.tile_pool(name="apsum", bufs=4, space="PSUM") as apool,
        tc.tile_pool(name="ppsum", bufs=2, space="PSUM") as ppool,
        tc.tile_pool(name="dpsum", bufs=1, space="PSUM") as dpool,
        tc.tile_pool(name="lpsum", bufs=1, space="PSUM") as lpool,
    ):
        def load(name, shape, dtype, src):
            t = const.tile(shape, dtype, tag=name, name=name)
            nc.sync.dma_start(t[:], src)
            return t

        w1z = load("w1z", [128, H], F16, din["w1zrep"][:, :])
        g = [load(f"g{i}", [H, H], F16, din["gmat"][i, :, :]) for i in range(2)]
        w2 = load("w2", [H, H], F16, din["w2"][:, :])
        cm = load("cm", [H, H], F16, din["cmat"][:, :])
        w3g = load("w3g", [H, 2 * D], F16, din["w3g"][:, :])
        rvec = load("rvec", [128, 4], F16, din["rvec"][:, :])
        ccol = load("ccol", [128, 1], F32, din["ccol"][:, :])
        btab = load("btab", [128, NSTEP * 4], F32, din["btab"][:, :])
        b2col = load("b2col", [128, 1], F32, din["b2col"][:, :])

        zpk = [state.tile([128, S], F32, tag=f"z{i}", name=f"z{i}") for i in range(2)]
        lpk = [state.tile([128, S], F32, tag=f"l{i}", name=f"l{i}") for i in range(2)]
        zf16 = [state.tile([128, S], F16, tag=f"zf{i}", name=f"zf{i}") for i in range(2)]
        nc.sync.dma_start(zpk[0][:], din["z0p"][:, :])
        nc.vector.memset(lpk[0][:], 0.0)
        nc.vector.tensor_copy(zf16[0][:], zpk[0][:])

        # gamma index per eval: dt/6 for evals 0,3 ; dt/3 for evals 1,2
        gidx = [0, 1, 1, 0]

        # persistent PSUM accumulators (strip rows are MM-written each step;
        # the other rows stay at the memset value so full-tile reads are defined)
        zdelta = dpool.tile([128, S], F32, tag="zd", name="zd")
        lpdelta = lpool.tile([128, S], F32, tag="lpd", name="lpd")
        nc.vector.memset(zdelta[:], 0.0)
        nc.vector.memset(lpdelta[:], 0.0)

        def steps():
            for s in range(NSTEP):
                run_step(s)

        def run_step(s):
            zsrc, zdst = zpk[s % 2], zpk[(s + 1) % 2]
            lsrc, ldst = lpk[s % 2], lpk[(s + 1) % 2]
            h2_prev = None
            pend = None  # deferred (gi, h1sq_t, h2_t, pw_t) from previous eval

            def flush_pend():
                gi_p, h1sq_p, h2_p, pw_p, e_p = pend
                for c in range(NCH):
                    nc.tensor.matmul(
                        zdelta[32 * c : 32 * c + D, :],
                        w3g[:, D * gi_p : D * gi_p + D],
                        h2_p[c][:],
                        start=(e_p == 0),
                        stop=(e_p == 3),
                        tile_position=(0, 32 * c),
                        skip_group_check=True,
                    )
                for c in range(NCH):
                    nc.tensor.matmul(
                        lpdelta[32 * c : 32 * c + 1, :],
                        rvec[:, gi_p : gi_p + 1],
                        h1sq_p[c][:],
                        start=(e_p == 0),
                        stop=False,
                        tile_position=(0, 32 * c),
                        skip_group_check=True,
                    )
                for c in range(NCH):
                    nc.tensor.matmul(
                        lpdelta[32 * c : 32 * c + 1, :],
                        rvec[:, 2 + gi_p : 3 + gi_p],
                        pw_p[c][:],
                        start=False,
                        stop=(e_p == 3),
                        tile_position=(0, 32 * c),
                        skip_group_check=True,
                    )

            for e in range(4):
                bcol = s * 4 + e
                a1_t, h1_t, a2_t, h1sq_t, h2_t, p_t, h2sq_t, pw_t = ([] for _ in range(8))
                for c in range(NCH):
                    a1 = apool.tile([128, S], F32, tag="a", name="a1")
                    nc.tensor.matmul(
                        a1[:],
                        w1z[32 * c : 32 * c + D, :],
                        zf16[s % 2][32 * c : 32 * c + D, :],
                        start=True,
                        stop=(e == 0),
                        tile_position=(32 * c, 0),
                    )
                    a1_t.append(a1)
                if e > 0:
                    # zvar coefficient: evals 1,2 use dt/2 (g[0]); eval 3 uses dt (g[1])
                    for c in range(NCH):
                        nc.tensor.matmul(
                            a1_t[c][:], g[0 if e < 3 else 1][:], h2_prev[c][:],
                            start=False, stop=True,
                        )
                if pend is not None:
                    flush_pend()
                for c in range(NCH):
                    h1 = h1p.tile([128, S], F16, tag="h1")
                    nc.scalar.activation(
                        h1[:], a1_t[c][:], AF.Tanh, bias=btab[:, bcol : bcol + 1]
                    )
                    h1_t.append(h1)
                for c in range(NCH):
                    a2 = apool.tile([128, S], F32, tag="a")
                    nc.tensor.matmul(a2[:], w2[:], h1_t[c][:], start=True, stop=True)
                    a2_t.append(a2)
                for c in range(NCH):
                    h1sq = h1sqp.tile([128, S], F16, tag="h1sq")
                    SQ1_ENGINE.tensor_mul(h1sq[:], h1_t[c][:], h1_t[c][:])
                    h1sq_t.append(h1sq)
                for c in range(NCH):
                    h2 = h2p.tile([128, S], F16, tag="h2")
                    nc.scalar.activation(
                        h2[:], a2_t[c][:], AF.Tanh, bias=b2col[:, 0:1]
                    )
                    h2_t.append(h2)
                for c in range(NCH):
                    p = ppool.tile([128, S], F32, tag="p")
                    nc.tensor.matmul(p[:], cm[:], h1sq_t[c][:], start=True, stop=True)
                    p_t.append(p)
                for c in range(NCH):
                    h2sq = h2sqp.tile([128, S], F16, tag="h2sq")
                    nc.vector.tensor_mul(h2sq[:], h2_t[c][:], h2_t[c][:])
                    h2sq_t.append(h2sq)
                for c in range(NCH):
                    pw = pwp.tile([128, S], F16, tag="pw")
                    nc.vector.scalar_tensor_tensor(
                        pw[:],
                        p_t[c][:],
                        ccol[:, 0:1],
                        h2sq_t[c][:],
                        op0=OP.subtract,
                        op1=OP.mult,
                    )
                    pw_t.append(pw)
                pend = (gidx[e], h1sq_t, h2_t, pw_t, e)
                h2_prev = h2_t
            flush_pend()
            nc.vector.tensor_add(zdst[:], zsrc[:], zdelta[:])
            nc.vector.tensor_copy(zf16[(s + 1) % 2][:], zdst[:])
            nc.vector.tensor_add(ldst[:], lsrc[:], lpdelta[:])
            nc.sync.dma_start(ztp[s, :, :], zdst[:])
            for c in range(NCH):
                nc.sync.dma_start(lpp[s, c : c + 1, :], ldst[32 * c : 32 * c + 1, :])

        if repeat == 1:
            steps()
        else:
            with tc.For_i(0, repeat, 1):
                steps()


def _host_prep(ts, z0, logp_diff_t0, W1, b1, W2, b2, W3, b3):
    """Precompute all per-core input tensors (float64 internally)."""
    f8 = np.float64
    ts = ts.astype(f8)
    W1, b1 = W1.astype(f8), b1.astype(f8)
    W2, b2 = W2.astype(f8), b2.astype(f8)
    W3, b3 = W3.astype(f8), b3.astype(f8)
    dts = np.diff(ts)
    assert np.allclose(dts, dts[0], rtol=0, atol=1e-9), "nonuniform ts unsupported"
    dt = float(dts[0])

    W1z = W1[:D, :]
    w1t = W1[D, :]
    B = W1z.T @ W3.T
    C = W2 * B
    G = W3 @ W1z
    c_rowsum = C.sum(axis=1)
    c_colsum = C.sum(axis=0)
    S0 = float(C.sum())
    ga, gb = dt / 6.0, dt / 3.0
    c_coefs = [0.0, dt / 2.0, dt / 2.0, dt]

    bf = np.float16
    w1zrep = np.zeros((128, H), np.float16)
    for c in range(NCH):
        w1zrep[32 * c : 32 * c + D, :] = W1z
    gmat = np.stack([(dt / 2.0) * G, dt * G]).astype(bf)        # [2,H,H]
    w3g = np.concatenate([ga * W3, gb * W3], axis=1).astype(bf)  # [H,16]
    rvec = np.stack(
        [ga * c_rowsum, gb * c_rowsum, -ga * np.ones(H), -gb * np.ones(H)], axis=1
    ).astype(bf)                                                 # [128,4]
    btab = np.zeros((128, NSTEP * 4), np.float32)
    w1tb3 = W1z.T @ b3
    for s in range(NSTEP):
        t0 = float(ts[s])
        tv = [t0, t0 + dt / 2.0, t0 + dt / 2.0, t0 + dt]
        for e in range(4):
            btab[:, s * 4 + e] = b1 + tv[e] * w1t + c_coefs[e] * w1tb3

    shared = dict(
        w1zrep=w1zrep,
        gmat=gmat,
        w2=W2.astype(bf),
        cmat=C.astype(bf),
        w3g=w3g,
        rvec=rvec,
        ccol=c_colsum.astype(np.float32).reshape(128, 1),
        btab=btab,
        b2col=b2.astype(np.float32).reshape(128, 1),
    )
    in_maps = []
    z0 = np.asarray(z0, np.float32)
    for core in range(NCORES):
        z0p = np.zeros((128, S), np.float32)
        for c in range(NCH):
            seg = z0[core * NSAMP + c * S : core * NSAMP + (c + 1) * S, :]  # [S,D]
            z0p[32 * c : 32 * c + D, :] = seg.T
        in_maps.append(dict(shared, z0p=z0p))
    return in_maps, dt, S0


def _assemble(results, z0, logp_diff_t0, dt, S0):
    z0 = np.asarray(z0, np.float32)
    lp0 = np.asarray(logp_diff_t0, np.float32)
    zt = np.empty((T, N, D), np.float32)
    lp = np.empty((T, N, 1), np.float32)
    zt[0] = z0
    lp[0] = lp0
    for core in range(NCORES):
        ztp = results[core]["ztp"]  # [NSTEP,128,S]
        lpp = results[core]["lpp"]  # [NSTEP,NCH,S]
        for c in range(NCH):
            sl = slice(core * NSAMP + c * S, core * NSAMP + (c + 1) * S)
            zt[1:, sl, :] = ztp[:, 32 * c : 32 * c + D, :].transpose(0, 2, 1)
            lp[1:, sl, 0] = lpp[:, c, :]
    for s in range(1, T):
        lp[s] += lp0 - np.float32(s * dt * S0)
    return zt, lp


def kernel(ts, z0, logp_diff_t0, W1, b1, W2, b2, W3, b3, _run=None):
    if "nc" not in _CACHE:
        _CACHE["nc"] = _build_nc()
    nc = _CACHE["nc"]
    in_maps, dt, S0 = _host_prep(ts, z0, logp_diff_t0, W1, b1, W2, b2, W3, b3)
    if _run is None:
        results = run_bass_kernel_spmd(nc, in_maps, core_ids=list(range(NCORES))).results
    else:
        results = _run(nc, in_maps)
    return _assemble(results, z0, logp_diff_t0, dt, S0)


# revision 25
# speedup vs baseline: 3269.4934x; 1.1674x over previous
"""Trainium2 Bass kernel for the CNF (continuous normalizing flow) problem.

reference math: RK4 integration (8 steps) of
    dz/dt = f(t,z) = MLP(concat[z, t]),  dlogp/dt = -tr(df/dz)
with MLP = tanh(W1x+b1) -> tanh(W2h+b2) -> W3h+b3, N=16384, D=8, H=128.

Key algebra (validated to fp32 accuracy on host):
  - exact Jacobian trace as a bilinear form:  tr = u^T C v  with
    u = 1-h1^2, v = 1-h2^2, C = W2 * (W1[:8].T @ W3.T)  (constant).
  - tr = S0 - c_rowsum.h1sq + sum_m[(P - c_colsum) * h2sq],  P = C^T h1sq,
    so only two partition-reduce matmuls (+ a fused DVE scalar_tensor_tensor)
    are needed per eval; the S0 constant is folded in on the host.
  - RK4 intermediate states are never materialized: the layer-1 preact for
    eval e is accumulated in PSUM as W1z^T z + c_e (W3@W1z)^T h2_{e-1},
    with (b1 + t*W1[8] + c_e W1z^T b3) applied as the tanh per-partition bias.

Layout: feature-major [features->partitions, samples->free]. Data-parallel
over N across 8 cores; per core 2048 samples = 4 chunks of 512 (PSUM bank
width). State z lives packed in one [128,512] tile, chunk c at partition
strip 32c (8 rows used per strip), enabling row/col-tiled small matmuls and
full-width elementwise ops for the tiny D=8 state. All matmul operands are
fp16 (same PE rate as bf16, 8x the mantissa; fp32 PSUM accumulate), state
and psum accumulators fp32. Elementwise work runs on VectorE + ScalarE only
(GPSIMD tensor ops measured ~5-10x slower than DVE on hardware).
"""

import numpy as np
import ml_dtypes

import concourse.bass as bass
import concourse.bacc as bacc
import concourse.tile as tile
import concourse.mybir as mybir
from concourse.bass_utils import run_bass_kernel_spmd

F32 = mybir.dt.float32
F32R = mybir.dt.float32r
F16 = mybir.dt.float16  # same PE/DVE speed as bf16, 8x more mantissa
AF = mybir.ActivationFunctionType
OP = mybir.AluOpType

N, D, H, T = 16384, 8, 128, 9
NCORES = 8
NSAMP = N // NCORES          # 2048 samples per core
S = 512                      # chunk width (one PSUM bank of fp32)
NCH = NSAMP // S             # 4 chunks per core
NSTEP = T - 1                # 8 RK4 steps

_CACHE = {}


def _build_nc(repeat=1):
    nc = bacc.Bacc("TRN2", target_bir_lowering=False, debug=False)

    din = {}
    din["z0p"] = nc.dram_tensor("z0p", [128, S], F32, kind="ExternalInput")
    din["w1zrep"] = nc.dram_tensor("w1zrep", [128, H], F16, kind="ExternalInput")
    din["gmat"] = nc.dram_tensor("gmat", [2, H, H], F16, kind="ExternalInput")
    din["w2"] = nc.dram_tensor("w2", [H, H], F16, kind="ExternalInput")
    din["cmat"] = nc.dram_tensor("cmat", [H, H], F16, kind="ExternalInput")
    din["w3g"] = nc.dram_tensor("w3g", [H, 2 * D], F16, kind="ExternalInput")
    din["rvec"] = nc.dram_tensor("rvec", [128, 4], F16, kind="ExternalInput")
    din["ccol"] = nc.dram_tensor("ccol", [128, 1], F32, kind="ExternalInput")
    din["btab"] = nc.dram_tensor("btab", [128, NSTEP * 4], F32, kind="ExternalInput")
    din["b2col"] = nc.dram_tensor("b2col", [128, 1], F32, kind="ExternalInput")
    ztp = nc.dram_tensor("ztp", [NSTEP, 128, S], F32, kind="ExternalOutput")
    lpp = nc.dram_tensor("lpp", [NSTEP, NCH, S], F32, kind="ExternalOutput")

    with tile.TileContext(nc) as tc:
        _body(nc, tc, din, ztp, lpp, repeat)
    nc.compile()
    return nc


def _body(nc, tc, din, ztp, lpp, repeat=1):
    # NB: GPSIMD tensor ops measured ~5-10x slower than DVE on HW - keep all
    # elementwise work on DVE (VectorE) and transcendentals on ScalarE.
    SQ1_ENGINE = nc.vector
    with (
        tc.tile_pool(name="const", bufs=1) as const,
        tc.tile_pool(name="state", bufs=1) as state,
        tc.tile_pool(name="h1", bufs=8) as h1p,
        tc.tile_pool(name="h1sq", bufs=8) as h1sqp,
        tc.tile_pool(name="h2", bufs=8) as h2p,
        tc.tile_pool(name="h2sq", bufs=8) as h2sqp,
        tc.tile_pool(name="pw", bufs=8) as pwp,
        tc.tile_pool(name="apsum", bufs=4, space="PSUM") as apool,
        tc.tile_pool(name="ppsum", bufs=2, space="PSUM") as ppool,
        tc.tile_pool(name="dpsum", bufs=1, space="PSUM") as dpool,
        tc.tile_pool(name="lpsum", bufs=1, space="PSUM") as lpool,
    ):
        def load(name, shape, dtype, src):
            t = const.tile(shape, dtype, tag=name, name=name)
            nc.sync.dma_start(t[:], src)
            return t

        w1z = load("w1z", [128, H], F16, din["w1zrep"][:, :])
        g = [load(f"g{i}", [H, H], F16, din["gmat"][i, :, :]) for i in range(2)]
        w2 = load("w2", [H, H], F16, din["w2"][:, :])
        cm = load("cm", [H, H], F16, din["cmat"][:, :])
        w3g = load("w3g", [H, 2 * D], F16, din["w3g"][:, :])
        rvec = load("rvec", [128, 4], F16, din["rvec"][:, :])
        ccol = load("ccol", [128, 1], F32, din["ccol"][:, :])
        btab = load("btab", [128, NSTEP * 4], F32, din["btab"][:, :])
        b2col = load("b2col", [128, 1], F32, din["b2col"][:, :])

        zpk = [state.tile([128, S], F32, tag=f"z{i}", name=f"z{i}") for i in range(2)]
        lpk = [state.tile([128, S], F32, tag=f"l{i}", name=f"l{i}") for i in range(2)]
        zf16 = [state.tile([128, S], F16, tag=f"zf{i}", name=f"zf{i}") for i in range(2)]
        nc.sync.dma_start(zpk[0][:], din["z0p"][:, :])
        nc.vector.memset(lpk[0][:], 0.0)
        nc.vector.tensor_copy(zf16[0][:], zpk[0][:])

        # gamma index per eval: dt/6 for evals 0,3 ; dt/3 for evals 1,2
        gidx = [0, 1, 1, 0]

        # persistent PSUM accumulators (strip rows are MM-written each step;
        # the other rows stay at the memset value so full-tile reads are defined)
        zdelta = dpool.tile([128, S], F32, tag="zd", name="zd")
        lpdelta = lpool.tile([128, S], F32, tag="lpd", name="lpd")
        nc.vector.memset(zdelta[:], 0.0)
        nc.vector.memset(lpdelta[:], 0.0)

        def steps():
            for s in range(NSTEP):
                run_step(s)

        def run_step(s):
            zsrc, zdst = zpk[s % 2], zpk[(s + 1) % 2]
            lsrc, ldst = lpk[s % 2], lpk[(s + 1) % 2]
            h2_prev = None
            pend = None  # deferred (gi, h1sq_t, h2_t, pw_t) from previous eval

            def flush_pend():
                gi_p, h1sq_p, h2_p, pw_p, e_p = pend
                for c in range(NCH):
                    nc.tensor.matmul(
                        zdelta[32 * c : 32 * c + D, :],
                        w3g[:, D * gi_p : D * gi_p + D],
                        h2_p[c][:],
                        start=(e_p == 0),
                        stop=(e_p == 3),
                        tile_position=(0, 32 * c),
                        skip_group_check=True,
                    )
                for c in range(NCH):
                    nc.tensor.matmul(
                        lpdelta[32 * c : 32 * c + 1, :],
                        rvec[:, gi_p : gi_p + 1],
                        h1sq_p[c][:],
                        start=(e_p == 0),
                        stop=False,
                        tile_position=(0, 32 * c),
                        skip_group_check=True,
                    )
                for c in range(NCH):
                    nc.tensor.matmul(
                        lpdelta[32 * c : 32 * c + 1, :],
                        rvec[:, 2 + gi_p : 3 + gi_p],
                        pw_p[c][:],
                        start=False,
                        stop=(e_p == 3),
                        tile_position=(0, 32 * c),
                        skip_group_check=True,
                    )

            for e in range(4):
                bcol = s * 4 + e
                a1_t, h1_t, a2_t, h1sq_t, h2_t, p_t, h2sq_t, pw_t = ([] for _ in range(8))
                for c in range(NCH):
                    a1 = apool.tile([128, S], F32, tag="a", name="a1")
                    nc.tensor.matmul(
                        a1[:],
                        w1z[32 * c : 32 * c + D, :],
                        zf16[s % 2][32 * c : 32 * c + D, :],
                        start=True,
                        stop=(e == 0),
                        tile_position=(32 * c, 0),
                    )
                    a1_t.append(a1)
                if e > 0:
                    # zvar coefficient: evals 1,2 use dt/2 (g[0]); eval 3 uses dt (g[1])
                    for c in range(NCH):
                        nc.tensor.matmul(
                            a1_t[c][:], g[0 if e < 3 else 1][:], h2_prev[c][:],
                            start=False, stop=True,
                        )
                if pend is not None:
                    flush_pend()
                for c in range(NCH):
                    h1 = h1p.tile([128, S], F16, tag="h1")
                    nc.scalar.activation(
                        h1[:], a1_t[c][:], AF.Tanh, bias=btab[:, bcol : bcol + 1]
                    )
                    h1_t.append(h1)
                for c in range(NCH):
                    a2 = apool.tile([128, S], F32, tag="a")
                    nc.tensor.matmul(a2[:], w2[:], h1_t[c][:], start=True, stop=True)
                    a2_t.append(a2)
                for c in range(NCH):
                    h1sq = h1sqp.tile([128, S], F16, tag="h1sq")
                    SQ1_ENGINE.tensor_mul(h1sq[:], h1_t[c][:], h1_t[c][:])
                    h1sq_t.append(h1sq)
                for c in range(NCH):
                    h2 = h2p.tile([128, S], F16, tag="h2")
                    nc.scalar.activation(
                        h2[:], a2_t[c][:], AF.Tanh, bias=b2col[:, 0:1]
                    )
                    h2_t.append(h2)
                for c in range(NCH):
                    p = ppool.tile([128, S], F32, tag="p")
                    nc.tensor.matmul(p[:], cm[:], h1sq_t[c][:], start=True, stop=True)
                    p_t.append(p)
                for c in range(NCH):
                    h2sq = h2sqp.tile([128, S], F16, tag="h2sq")
                    if c == 3:
                        nc.scalar.square(h2sq[:], h2_t[c][:])
                    else:
                        nc.vector.tensor_mul(h2sq[:], h2_t[c][:], h2_t[c][:])
                    h2sq_t.append(h2sq)
                for c in range(NCH):
                    pw = pwp.tile([128, S], F16, tag="pw")
                    nc.vector.scalar_tensor_tensor(
                        pw[:],
                        p_t[c][:],
                        ccol[:, 0:1],
                        h2sq_t[c][:],
                        op0=OP.subtract,
                        op1=OP.mult,
                    )
                    pw_t.append(pw)
                pend = (gidx[e], h1sq_t, h2_t, pw_t, e)
                h2_prev = h2_t
            flush_pend()
            nc.vector.tensor_add(zf16[(s + 1) % 2][:], zsrc[:], zdelta[:])
            nc.vector.tensor_add(zdst[:], zsrc[:], zdelta[:])
            nc.vector.tensor_add(ldst[:], lsrc[:], lpdelta[:])
            nc.sync.dma_start(ztp[s, :, :], zdst[:])
            for c in range(NCH):
                nc.sync.dma_start(lpp[s, c : c + 1, :], ldst[32 * c : 32 * c + 1, :])

        if repeat == 1:
            steps()
        else:
            with tc.For_i(0, repeat, 1):
                steps()


def _host_prep(ts, z0, logp_diff_t0, W1, b1, W2, b2, W3, b3):
    """Precompute all per-core input tensors (float64 internally)."""
    f8 = np.float64
    ts = ts.astype(f8)
    W1, b1 = W1.astype(f8), b1.astype(f8)
    W2, b2 = W2.astype(f8), b2.astype(f8)
    W3, b3 = W3.astype(f8), b3.astype(f8)
    dts = np.diff(ts)
    assert np.allclose(dts, dts[0], rtol=0, atol=1e-9), "nonuniform ts unsupported"
    dt = float(dts[0])

    W1z = W1[:D, :]
    w1t = W1[D, :]
    B = W1z.T @ W3.T
    C = W2 * B
    G = W3 @ W1z
    c_rowsum = C.sum(axis=1)
    c_colsum = C.sum(axis=0)
    S0 = float(C.sum())
    ga, gb = dt / 6.0, dt / 3.0
    c_coefs = [0.0, dt / 2.0, dt / 2.0, dt]

    bf = np.float16
    w1zrep = np.zeros((128, H), np.float16)
    for c in range(NCH):
        w1zrep[32 * c : 32 * c + D, :] = W1z
    gmat = np.stack([(dt / 2.0) * G, dt * G]).astype(bf)        # [2,H,H]
    w3g = np.concatenate([ga * W3, gb * W3], axis=1).astype(bf)  # [H,16]
    rvec = np.stack(
        [ga * c_rowsum, gb * c_rowsum, -ga * np.ones(H), -gb * np.ones(H)], axis=1
    ).astype(bf)                                                 # [128,4]
    btab = np.zeros((128, NSTEP * 4), np.float32)
    w1tb3 = W1z.T @ b3
    for s in range(NSTEP):
        t0 = float(ts[s])
        tv = [t0, t0 + dt / 2.0, t0 + dt / 2.0, t0 + dt]
        for e in range(4):
            btab[:, s * 4 + e] = b1 + tv[e] * w1t + c_coefs[e] * w1tb3

    shared = dict(
        w1zrep=w1zrep,
        gmat=gmat,
        w2=W2.astype(bf),
        cmat=C.astype(bf),
        w3g=w3g,
        rvec=rvec,
        ccol=c_colsum.astype(np.float32).reshape(128, 1),
        btab=btab,
        b2col=b2.astype(np.float32).reshape(128, 1),
    )
    in_maps = []
    z0 = np.asarray(z0, np.float32)
    for core in range(NCORES):
        z0p = np.zeros((128, S), np.float32)
        for c in range(NCH):
            seg = z0[core * NSAMP + c * S : core * NSAMP + (c + 1) * S, :]  # [S,D]
            z0p[32 * c : 32 * c + D, :] = seg.T
        in_maps.append(dict(shared, z0p=z0p))
    return in_maps, dt, S0


def _assemble(results, z0, logp_diff_t0, dt, S0):
    z0 = np.asarray(z0, np.float32)
    lp0 = np.asarray(logp_diff_t0, np.float32)
    zt = np.empty((T, N, D), np.float32)
    lp = np.empty((T, N, 1), np.float32)
    zt[0] = z0
    lp[0] = lp0
    for core in range(NCORES):
        ztp = results[core]["ztp"]  # [NSTEP,128,S]
        lpp = results[core]["lpp"]  # [NSTEP,NCH,S]
        for c in range(NCH):
            sl = slice(core * NSAMP + c * S, core * NSAMP + (c + 1) * S)
            zt[1:, sl, :] = ztp[:, 32 * c : 32 * c + D, :].transpose(0, 2, 1)
            lp[1:, sl, 0] = lpp[:, c, :]
    for s in range(1, T):
        lp[s] += lp0 - np.float32(s * dt * S0)
    return zt, lp


def kernel(ts, z0, logp_diff_t0, W1, b1, W2, b2, W3, b3, _run=None):
    ts, z0, logp_diff_t0 = np.asarray(ts), np.asarray(z0), np.asarray(logp_diff_t0)
    W1, b1, W2, b2 = np.asarray(W1), np.asarray(b1), np.asarray(W2), np.asarray(b2)
    W3, b3 = np.asarray(W3), np.asarray(b3)
    if "nc" not in _CACHE:
        _CACHE["nc"] = _build_nc()
    nc = _CACHE["nc"]
    in_maps, dt, S0 = _host_prep(ts, z0, logp_diff_t0, W1, b1, W2, b2, W3, b3)
    if _run is None:
        results = run_bass_kernel_spmd(nc, in_maps, core_ids=list(range(NCORES))).results
    else:
        results = _run(nc, in_maps)
    return _assemble(results, z0, logp_diff_t0, dt, S0)


# revision 27
# speedup vs baseline: 3722.2938x; 1.1385x over previous
"""Trainium2 Bass kernel for the CNF (continuous normalizing flow) problem.

reference math: RK4 integration (8 steps) of
    dz/dt = f(t,z) = MLP(concat[z, t]),  dlogp/dt = -tr(df/dz)
with MLP = tanh(W1x+b1) -> tanh(W2h+b2) -> W3h+b3, N=16384, D=8, H=128.

Key algebra (validated to fp32 accuracy on host):
  - exact Jacobian trace as a bilinear form:  tr = u^T C v  with
    u = 1-h1^2, v = 1-h2^2, C = W2 * (W1[:8].T @ W3.T)  (constant).
  - tr = S0 - c_rowsum.h1sq + sum_m[(P - c_colsum) * h2sq],  P = C^T h1sq,
    so only two partition-reduce matmuls (+ a fused DVE scalar_tensor_tensor)
    are needed per eval; the S0 constant is folded in on the host.
  - RK4 intermediate states are never materialized: the layer-1 preact for
    eval e is accumulated in PSUM as W1z^T z + c_e (W3@W1z)^T h2_{e-1},
    with (b1 + t*W1[8] + c_e W1z^T b3) applied as the tanh per-partition bias.

Layout: feature-major [features->partitions, samples->free]. Data-parallel
over N across 8 cores; per core 2048 samples = 4 chunks of 512 (PSUM bank
width). State z lives packed in one [128,512] tile, chunk c at partition
strip 32c (8 rows used per strip), enabling row/col-tiled small matmuls and
full-width elementwise ops for the tiny D=8 state. All matmul operands are
fp16 (same PE rate as bf16, 8x the mantissa; fp32 PSUM accumulate), state
and psum accumulators fp32. Elementwise work runs on VectorE + ScalarE only
(GPSIMD tensor ops measured ~5-10x slower than DVE on hardware).
"""

import numpy as np
import ml_dtypes

import concourse.bass as bass
import concourse.bacc as bacc
import concourse.tile as tile
import concourse.mybir as mybir
from concourse.bass_utils import run_bass_kernel_spmd

F32 = mybir.dt.float32
F32R = mybir.dt.float32r
F16 = mybir.dt.float16  # same PE/DVE speed as bf16, 8x more mantissa
AF = mybir.ActivationFunctionType
OP = mybir.AluOpType

N, D, H, T = 16384, 8, 128, 9
NCORES = 8
NSAMP = N // NCORES          # 2048 samples per core
S = 512                      # chunk width (one PSUM bank of fp32)
NCH = NSAMP // S             # 4 chunks per core
NSTEP = T - 1                # 8 RK4 steps

_CACHE = {}


def _build_nc(repeat=1):
    nc = bacc.Bacc("TRN2", target_bir_lowering=False, debug=False)

    din = {}
    din["z0p"] = nc.dram_tensor("z0p", [128, S], F32, kind="ExternalInput")
    din["w1zrep"] = nc.dram_tensor("w1zrep", [128, H], F16, kind="ExternalInput")
    din["gmat"] = nc.dram_tensor("gmat", [2, H, H], F16, kind="ExternalInput")
    din["w2"] = nc.dram_tensor("w2", [H, H], F16, kind="ExternalInput")
    din["cmat"] = nc.dram_tensor("cmat", [H, H], F16, kind="ExternalInput")
    din["w3g"] = nc.dram_tensor("w3g", [H, 2 * D], F16, kind="ExternalInput")
    din["rvec"] = nc.dram_tensor("rvec", [128, 4], F16, kind="ExternalInput")
    din["ccol"] = nc.dram_tensor("ccol", [128, 1], F32, kind="ExternalInput")
    din["btab"] = nc.dram_tensor("btab", [128, NSTEP * 4], F32, kind="ExternalInput")
    din["b2col"] = nc.dram_tensor("b2col", [128, 1], F32, kind="ExternalInput")
    ztp = nc.dram_tensor("ztp", [NSTEP, 128, S], F32, kind="ExternalOutput")
    lpp = nc.dram_tensor("lpp", [NSTEP, NCH, S], F32, kind="ExternalOutput")

    with tile.TileContext(nc) as tc:
        _body(nc, tc, din, ztp, lpp, repeat)
    nc.compile()
    return nc


def _body(nc, tc, din, ztp, lpp, repeat=1):
    # NB: GPSIMD tensor ops measured ~5-10x slower than DVE on HW - keep all
    # elementwise work on DVE (VectorE) and transcendentals on ScalarE.
    SQ1_ENGINE = nc.vector
    with (
        tc.tile_pool(name="const", bufs=1) as const,
        tc.tile_pool(name="state", bufs=1) as state,
        tc.tile_pool(name="h1", bufs=8) as h1p,
        tc.tile_pool(name="h1sq", bufs=8) as h1sqp,
        tc.tile_pool(name="h2", bufs=8) as h2p,
        tc.tile_pool(name="h2sq", bufs=8) as h2sqp,
        tc.tile_pool(name="pw", bufs=8) as pwp,
        tc.tile_pool(name="apsum", bufs=4, space="PSUM") as apool,
        tc.tile_pool(name="ppsum", bufs=2, space="PSUM") as ppool,
        tc.tile_pool(name="dpsum", bufs=1, space="PSUM") as dpool,
        tc.tile_pool(name="lpsum", bufs=1, space="PSUM") as lpool,
    ):
        def load(name, shape, dtype, src):
            t = const.tile(shape, dtype, tag=name, name=name)
            nc.sync.dma_start(t[:], src)
            return t

        w1z = load("w1z", [128, H], F16, din["w1zrep"][:, :])
        g = [load(f"g{i}", [H, H], F16, din["gmat"][i, :, :]) for i in range(2)]
        w2 = load("w2", [H, H], F16, din["w2"][:, :])
        cm = load("cm", [H, H], F16, din["cmat"][:, :])
        w3g = load("w3g", [H, 2 * D], F16, din["w3g"][:, :])
        rvec = load("rvec", [128, 4], F16, din["rvec"][:, :])
        ccol = load("ccol", [128, 1], F32, din["ccol"][:, :])
        btab = load("btab", [128, NSTEP * 4], F32, din["btab"][:, :])
        b2col = load("b2col", [128, 1], F32, din["b2col"][:, :])

        zpk = [state.tile([128, S], F32, tag=f"z{i}", name=f"z{i}") for i in range(2)]
        lpk = [state.tile([128, S], F32, tag=f"l{i}", name=f"l{i}") for i in range(2)]
        zf16 = [state.tile([128, S], F16, tag=f"zf{i}", name=f"zf{i}") for i in range(2)]
        nc.sync.dma_start(zpk[0][:], din["z0p"][:, :])
        nc.vector.memset(lpk[0][:], 0.0)
        nc.vector.tensor_copy(zf16[0][:], zpk[0][:])
        # trigger the Tanh table-set load while weight DMAs are in flight
        actwarm = const.tile([128, 1], F16, tag="actwarm", name="actwarm")
        nc.scalar.activation(actwarm[:], b2col[:, 0:1], AF.Tanh)

        # gamma index per eval: dt/6 for evals 0,3 ; dt/3 for evals 1,2
        gidx = [0, 1, 1, 0]

        # persistent PSUM accumulators (strip rows are MM-written each step;
        # the other rows stay at the memset value so full-tile reads are defined)
        zdelta = dpool.tile([128, S], F32, tag="zd", name="zd")
        lpdelta = lpool.tile([128, S], F32, tag="lpd", name="lpd")
        nc.vector.memset(zdelta[:], 0.0)
        nc.vector.memset(lpdelta[:], 0.0)

        def steps():
            for s in range(NSTEP):
                run_step(s)

        def run_step(s):
            zsrc, zdst = zpk[s % 2], zpk[(s + 1) % 2]
            lsrc, ldst = lpk[s % 2], lpk[(s + 1) % 2]
            h2_prev = None
            pend = None  # deferred (gi, h1sq_t, h2_t, pw_t) from previous eval

            def flush_pend():
                gi_p, h1sq_p, h2_p, pw_p, e_p = pend
                for c in range(NCH):
                    nc.tensor.matmul(
                        zdelta[32 * c : 32 * c + D, :],
                        w3g[:, D * gi_p : D * gi_p + D],
                        h2_p[c][:],
                        start=(e_p == 0),
                        stop=(e_p == 3),
                        tile_position=(0, 32 * c),
                        skip_group_check=True,
                    )
                for c in range(NCH):
                    nc.tensor.matmul(
                        lpdelta[32 * c : 32 * c + 1, :],
                        rvec[:, gi_p : gi_p + 1],
                        h1sq_p[c][:],
                        start=(e_p == 0),
                        stop=False,
                        tile_position=(0, 32 * c),
                        skip_group_check=True,
                    )
                for c in range(NCH):
                    nc.tensor.matmul(
                        lpdelta[32 * c : 32 * c + 1, :],
                        rvec[:, 2 + gi_p : 3 + gi_p],
                        pw_p[c][:],
                        start=False,
                        stop=(e_p == 3),
                        tile_position=(0, 32 * c),
                        skip_group_check=True,
                    )

            for e in range(4):
                bcol = s * 4 + e
                a1_t, h1_t, a2_t, h1sq_t, h2_t, p_t, h2sq_t, pw_t = ([] for _ in range(8))
                for c in range(NCH):
                    a1 = apool.tile([128, S], F32, tag="a", name="a1")
                    nc.tensor.matmul(
                        a1[:],
                        w1z[32 * c : 32 * c + D, :],
                        zf16[s % 2][32 * c : 32 * c + D, :],
                        start=True,
                        stop=(e == 0),
                        tile_position=(32 * c, 0),
                    )
                    a1_t.append(a1)
                if e > 0:
                    # zvar coefficient: evals 1,2 use dt/2 (g[0]); eval 3 uses dt (g[1])
                    for c in range(NCH):
                        nc.tensor.matmul(
                            a1_t[c][:], g[0 if e < 3 else 1][:], h2_prev[c][:],
                            start=False, stop=True,
                        )
                if pend is not None:
                    flush_pend()
                for c in range(NCH):
                    h1 = h1p.tile([128, S], F16, tag="h1")
                    nc.scalar.activation(
                        h1[:], a1_t[c][:], AF.Tanh, bias=btab[:, bcol : bcol + 1]
                    )
                    h1_t.append(h1)
                for c in range(NCH):
                    a2 = apool.tile([128, S], F32, tag="a")
                    nc.tensor.matmul(a2[:], w2[:], h1_t[c][:], start=True, stop=True)
                    a2_t.append(a2)
                for c in range(NCH):
                    h1sq = h1sqp.tile([128, S], F16, tag="h1sq")
                    SQ1_ENGINE.tensor_mul(h1sq[:], h1_t[c][:], h1_t[c][:])
                    h1sq_t.append(h1sq)
                for c in range(NCH):
                    h2 = h2p.tile([128, S], F16, tag="h2")
                    nc.scalar.activation(
                        h2[:], a2_t[c][:], AF.Tanh, bias=b2col[:, 0:1]
                    )
                    h2_t.append(h2)
                for c in range(NCH):
                    p = ppool.tile([128, S], F32, tag="p")
                    nc.tensor.matmul(p[:], cm[:], h1sq_t[c][:], start=True, stop=True)
                    p_t.append(p)
                for c in range(NCH):
                    h2sq = h2sqp.tile([128, S], F16, tag="h2sq")
                    if c == 3:
                        nc.scalar.square(h2sq[:], h2_t[c][:])
                    else:
                        nc.vector.tensor_mul(h2sq[:], h2_t[c][:], h2_t[c][:])
                    h2sq_t.append(h2sq)
                for c in range(NCH):
                    pw = pwp.tile([128, S], F16, tag="pw")
                    nc.vector.scalar_tensor_tensor(
                        pw[:],
                        p_t[c][:],
                        ccol[:, 0:1],
                        h2sq_t[c][:],
                        op0=OP.subtract,
                        op1=OP.mult,
                    )
                    pw_t.append(pw)
                pend = (gidx[e], h1sq_t, h2_t, pw_t, e)
                h2_prev = h2_t
            flush_pend()
            nc.vector.tensor_add(zf16[(s + 1) % 2][:], zsrc[:], zdelta[:])
            nc.vector.tensor_add(zdst[:], zsrc[:], zdelta[:])
            nc.vector.tensor_add(ldst[:], lsrc[:], lpdelta[:])
            nc.sync.dma_start(ztp[s, :, :], zdst[:])
            for c in range(NCH):
                nc.sync.dma_start(lpp[s, c : c + 1, :], ldst[32 * c : 32 * c + 1, :])

        if repeat == 1:
            steps()
        else:
            with tc.For_i(0, repeat, 1):
                steps()


def _host_prep(ts, z0, logp_diff_t0, W1, b1, W2, b2, W3, b3):
    """Precompute all per-core input tensors (float64 internally)."""
    f8 = np.float64
    ts = ts.astype(f8)
    W1, b1 = W1.astype(f8), b1.astype(f8)
    W2, b2 = W2.astype(f8), b2.astype(f8)
    W3, b3 = W3.astype(f8), b3.astype(f8)
    dts = np.diff(ts)
    assert np.allclose(dts, dts[0], rtol=0, atol=1e-9), "nonuniform ts unsupported"
    dt = float(dts[0])

    W1z = W1[:D, :]
    w1t = W1[D, :]
    B = W1z.T @ W3.T
    C = W2 * B
    G = W3 @ W1z
    c_rowsum = C.sum(axis=1)
    c_colsum = C.sum(axis=0)
    S0 = float(C.sum())
    ga, gb = dt / 6.0, dt / 3.0
    c_coefs = [0.0, dt / 2.0, dt / 2.0, dt]

    bf = np.float16
    w1zrep = np.zeros((128, H), np.float16)
    for c in range(NCH):
        w1zrep[32 * c : 32 * c + D, :] = W1z
    gmat = np.stack([(dt / 2.0) * G, dt * G]).astype(bf)        # [2,H,H]
    w3g = np.concatenate([ga * W3, gb * W3], axis=1).astype(bf)  # [H,16]
    rvec = np.stack(
        [ga * c_rowsum, gb * c_rowsum, -ga * np.ones(H), -gb * np.ones(H)], axis=1
    ).astype(bf)                                                 # [128,4]
    btab = np.zeros((128, NSTEP * 4), np.float32)
    w1tb3 = W1z.T @ b3
    for s in range(NSTEP):
        t0 = float(ts[s])
        tv = [t0, t0 + dt / 2.0, t0 + dt / 2.0, t0 + dt]
        for e in range(4):
            btab[:, s * 4 + e] = b1 + tv[e] * w1t + c_coefs[e] * w1tb3

    shared = dict(
        w1zrep=w1zrep,
        gmat=gmat,
        w2=W2.astype(bf),
        cmat=C.astype(bf),
        w3g=w3g,
        rvec=rvec,
        ccol=c_colsum.astype(np.float32).reshape(128, 1),
        btab=btab,
        b2col=b2.astype(np.float32).reshape(128, 1),
    )
    in_maps = []
    z0 = np.asarray(z0, np.float32)
    for core in range(NCORES):
        z0p = np.zeros((128, S), np.float32)
        for c in range(NCH):
            seg = z0[core * NSAMP + c * S : core * NSAMP + (c + 1) * S, :]  # [S,D]
            z0p[32 * c : 32 * c + D, :] = seg.T
        in_maps.append(dict(shared, z0p=z0p))
    return in_maps, dt, S0


def _assemble(results, z0, logp_diff_t0, dt, S0):
    z0 = np.asarray(z0, np.float32)
    lp0 = np.asarray(logp_diff_t0, np.float32)
    zt = np.empty((T, N, D), np.float32)
    lp = np.empty((T, N, 1), np.float32)
    zt[0] = z0
    lp[0] = lp0
    for core in range(NCORES):
        ztp = results[core]["ztp"]  # [NSTEP,128,S]
        lpp = results[core]["lpp"]  # [NSTEP,NCH,S]
        for c in range(NCH):
            sl = slice(core * NSAMP + c * S, core * NSAMP + (c + 1) * S)
            zt[1:, sl, :] = ztp[:, 32 * c : 32 * c + D, :].transpose(0, 2, 1)
            lp[1:, sl, 0] = lpp[:, c, :]
    for s in range(1, T):
        lp[s] += lp0 - np.float32(s * dt * S0)
    return zt, lp


def kernel(ts, z0, logp_diff_t0, W1, b1, W2, b2, W3, b3, _run=None):
    ts, z0, logp_diff_t0 = np.asarray(ts), np.asarray(z0), np.asarray(logp_diff_t0)
    W1, b1, W2, b2 = np.asarray(W1), np.asarray(b1), np.asarray(W2), np.asarray(b2)
    W3, b3 = np.asarray(W3), np.asarray(b3)
    if "nc" not in _CACHE:
        _CACHE["nc"] = _build_nc()
    nc = _CACHE["nc"]
    in_maps, dt, S0 = _host_prep(ts, z0, logp_diff_t0, W1, b1, W2, b2, W3, b3)
    if _run is None:
        results = run_bass_kernel_spmd(nc, in_maps, core_ids=list(range(NCORES))).results
    else:
        results = _run(nc, in_maps)
    return _assemble(results, z0, logp_diff_t0, dt, S0)
